# revision 60
# baseline (speedup 1.0000x reference)
"""LightGCN layer on 8 TRN2 NeuronCores.

out[r] = dis[r] * sum_{e: row_e = r} dis[col_e] * x[col_e]
where dis = masked rsqrt of destination-degree (deg = bincount(row)).

Strategy (edges sharded by destination chunk; xhat replicated):
- Host computes deg/dis from edge_index alone and ships xhat = dis*x as bf16
  rows padded to 256 bytes (dma_gather needs 256B-aligned elements).
- Edges are grouped by 128-row destination chunk; chunks are dealt to the 8
  cores balancing per-bank totals, then each core's chunks are ordered so the
  8 cores' per-bank cumulative-edge profiles stay aligned (the SPMD schedule
  is shared, so stream lengths and windows are maxima/unions over cores).
- Per core, edges form 4 bank-major streams (int16 gather indices address one
  of four 32768-row banks of xhat), each stream chunk-major sorted and packed
  back-to-back with padding only at the stream tail. dma_gather instructions
  of up to 8192 indices (single_packet=False streams descriptors through the
  SWDGE ring, lifting the 1024-idx single-packet cap; verified on hardware)
  chop each stream with no alignment waste, amortizing the ~1us-per-
  instruction SWDGE overhead that otherwise dominates. Stream tails use small
  instructions so the last chunks aren't gated by an 8192-descriptor DMA.
- Subtiles (128 slots) may straddle chunk boundaries, so the plan emits one
  matmul per (stream, subtile, candidate chunk), the candidate window being
  the union over cores of chunks overlapping that subtile. One-hots come from
  tensor_scalar is_equal against host-packed per-build row-lo columns; a
  subtile's two-chunk window is built as a single [128,256] one-hot against a
  256-wide iota (row-lo offset by 128 for the second chunk; 65535 marks edges
  outside the build, including pads). Single-chunk builds are split across
  DVE (is_equal), GpSimd (is_equal), and Activation (Relu(1-|iota-r|), exact
  on integers) to balance per-chunk engine load; all feed the PE. Chunk
  segment sums accumulate in PSUM; the Activation engine scales by dis[row]
  into a whole-core staging buffer (lagging a couple chunks so copies never
  head-of-line-block Act's builds), stored in a few large DMAs.
- Host scatters the 8 shards back through the chunk permutation and trims.
"""
import sys

try:
    import concourse  # noqa: F401  (provided by the booted axon site)
except ImportError:
    sys.path.insert(0, "/opt/trn_rl_repo")

import numpy as np
import ml_dtypes

N_NODES = 100000
N_EDGES = 1600000
D = 64
P = 128                       # edges per subtile (matmul contraction dim)
C = 128                       # destination rows per chunk (PSUM tile height)
XW = 128                      # padded xhat row width (bf16 -> 256B elements)
NCORE = 8
CPC = 98                      # chunks per core
NCHUNK = NCORE * CPC          # 784 chunks of 128 rows
NPAD = NCHUNK * C             # 100352 padded node count
BANKS = 4
BKSZ = 32768                  # int16-addressable rows per bank
NI_BIG = 64                   # subtiles per big dma_gather (8192 idx)
NI_TAIL = 16                  # subtile granularity for stream head/tail
TAIL_SUBS = 80                # final subtiles per stream chopped at NI_TAIL
FRONT_SUBS = 32               # leading subtiles per stream chopped at NI_TAIL
PAD_RLO = 65535               # one-hot-miss marker (u16)
LOOKAHEAD = 6                 # chunks of gather runahead in the issue order
OHB = 8                       # builds per one-hot tile (amortizes WAR sems)
BUILD_AHEAD = 4               # chunks of one-hot build runahead
BUILD_W = {"d": 0.69, "p": 0.135, "a": 0.175}  # single-build engine split


class _Plan:
    """Static shared schedule: streams, gather instructions, matmul plan."""

    def __init__(self, nsub, iwoff, n_builds, bwidth, entries, schedule):
        self.nsub = nsub          # [BANKS] subtiles per stream
        self.iwoff = iwoff        # [BANKS] stream offset in subtiles
        self.n_builds = n_builds
        self.bwidth = bwidth      # [n_builds] chunks covered (1 or 2)
        self.entries = entries    # per loc j: [(b, s, build, half), ...]
        self.schedule = schedule  # [("g", b, o, n) | ("c", j), ...]


def _deal_chunks(cnt4):
    """Assign chunks to cores (98 each) balancing the four per-bank stream
    totals, then order each core's chunks so cumulative profiles align."""
    tot = cnt4.sum(1)
    order = np.argsort(-tot, kind="stable")
    i = np.arange(NCHUNK)
    rnd, pos = i // NCORE, i % NCORE
    corei = np.where(rnd % 2 == 0, pos, NCORE - 1 - pos)
    core_of = np.empty(NCHUNK, np.int64)
    core_of[order] = corei

    lens = np.zeros((NCORE, BANKS), np.int64)
    for c in range(NCHUNK):
        lens[core_of[c]] += cnt4[c]

    def score(ls):
        return int((-(-ls.max(axis=0) // P)).sum()) * P + int(ls.max(axis=0).sum())

    chunks_of = [list(np.nonzero(core_of == k)[0]) for k in range(NCORE)]
    for _ in range(4):
        improved = False
        base = score(lens)
        for k1 in range(NCORE):
            for k2 in range(k1 + 1, NCORE):
                for c1 in chunks_of[k1][:]:
                    done = False
                    for c2 in chunks_of[k2][:]:
                        lens[k1] += cnt4[c2] - cnt4[c1]
                        lens[k2] += cnt4[c1] - cnt4[c2]
                        new = score(lens)
                        if new < base:
                            base = new
                            improved = True
                            chunks_of[k1].remove(c1)
                            chunks_of[k2].remove(c2)
                            chunks_of[k1].append(c2)
                            chunks_of[k2].append(c1)
                            core_of[c1], core_of[c2] = k2, k1
                            done = True
                            break
                        lens[k1] -= cnt4[c2] - cnt4[c1]
                        lens[k2] -= cnt4[c1] - cnt4[c2]
                    if done:
                        break
        if not improved:
            break

    loc_of = np.empty(NCHUNK, np.int64)
    grand = cnt4.sum(0).astype(np.float64)
    cum = np.zeros((NCORE, BANKS), np.float64)
    remaining = [set(chunks_of[k]) for k in range(NCORE)]
    for t in range(CPC):
        target = grand / NCORE * (t + 1) / CPC
        for k in range(NCORE):
            best, bestc = None, None
            for c in remaining[k]:
                dev = np.abs(cum[k] + cnt4[c] - target).max()
                if best is None or dev < best:
                    best, bestc = dev, c
            remaining[k].remove(bestc)
            loc_of[bestc] = t
            cum[k] += cnt4[bestc]
    return core_of, loc_of


def _chop_stream(ns):
    """Instruction sizes for a stream of ns subtiles: NI_BIG pieces, with the
    first FRONT_SUBS and last TAIL_SUBS subtiles at NI_TAIL granularity so
    the pipeline fills fast and the final chunks aren't gated behind one huge
    DMA."""
    out = []
    o = 0
    while o < min(FRONT_SUBS, ns):
        n = min(NI_TAIL, ns - o)
        out.append((o, n))
        o += n
    while ns - o > NI_BIG + TAIL_SUBS:
        out.append((o, NI_BIG))
        o += NI_BIG
    rem = ns - o
    if rem > TAIL_SUBS:
        out.append((o, rem - TAIL_SUBS))
        o += rem - TAIL_SUBS
    while ns > o:
        n = min(NI_TAIL, ns - o)
        out.append((o, n))
        o += n
    return out


def _pack_edges(row, col):
    chunk = (row >> 7).astype(np.int64)
    bank = (col >> 15).astype(np.int64)
    cnt4 = np.bincount(chunk * BANKS + bank,
                       minlength=NCHUNK * BANKS).reshape(NCHUNK, BANKS)
    core_of, loc_of = _deal_chunks(cnt4)

    cnt_kbj = np.zeros((NCORE, BANKS, CPC), np.int64)
    cnt_kbj[core_of[:, None], np.arange(BANKS)[None, :],
            loc_of[:, None]] = cnt4
    cum = np.zeros((NCORE, BANKS, CPC + 1), np.int64)
    np.cumsum(cnt_kbj, axis=2, out=cum[:, :, 1:])
    stream_len = cum[:, :, -1]                            # [NCORE, BANKS]
    nsub = np.asarray(-(-stream_len.max(axis=0) // P))    # [BANKS]
    iwoff = np.zeros(BANKS, np.int64)
    np.cumsum(nsub[:-1], out=iwoff[1:])
    nsub_tot = int(nsub.sum())

    # candidate windows per (bank, subtile): union over cores of locs
    # overlapping [P*s, P*(s+1)) within that core's stream
    w0 = [np.zeros(int(nsub[b]), np.int64) for b in range(BANKS)]
    wn = [np.zeros(int(nsub[b]), np.int64) for b in range(BANKS)]
    for b in range(BANKS):
        ns = int(nsub[b])
        lo = np.full(ns, CPC, np.int64)
        hi = np.full(ns, -1, np.int64)
        for k in range(NCORE):
            tot = int(stream_len[k, b])
            if tot == 0:
                continue
            nsk = -(-tot // P)
            s = np.arange(nsk)
            lo_k = np.searchsorted(cum[k, b, 1:], s * P, side="right")
            last = np.minimum((s + 1) * P, tot) - 1
            hi_k = np.searchsorted(cum[k, b, 1:], last, side="right")
            lo[:nsk] = np.minimum(lo[:nsk], lo_k)
            hi[:nsk] = np.maximum(hi[:nsk], hi_k)
        empty = hi < lo
        lo[empty], hi[empty] = 0, 0
        w0[b], wn[b] = lo, hi - lo + 1

    # builds: each covers up to 2 consecutive window chunks of a subtile.
    # bbase[b][s] = first build id of subtile s; entry (loc j) maps to
    # build bbase + (j-w0)//2, half (j-w0)%2.
    bbase = [np.zeros(int(nsub[b]), np.int64) for b in range(BANKS)]
    n_builds = 0
    for b in range(BANKS):
        nb_s = -(-wn[b] // 2)
        bbase[b][0] = n_builds
        if int(nsub[b]) > 1:
            np.cumsum(nb_s[:-1], out=bbase[b][1:])
            bbase[b][1:] += n_builds
        n_builds += int(nb_s.sum())

    entries = [[] for _ in range(CPC)]
    bwidth = np.ones(n_builds, np.int64)
    for b in range(BANKS):
        for s in range(int(nsub[b])):
            for w in range(int(wn[b][s])):
                j = int(w0[b][s]) + w
                bld = int(bbase[b][s]) + w // 2
                entries[j].append((b, s, bld, w % 2))
                if w % 2 == 1:
                    bwidth[bld] = 2
    for j in range(CPC):
        entries[j].sort()
        assert entries[j], f"loc {j} has no plan entries"

    # per-edge placement: rank within (core, bank) stream, chunk-major
    k_e = core_of[chunk]
    j_e = loc_of[chunk]
    key = (k_e * BANKS + bank) * CPC + j_e
    eorder = np.argsort(key, kind="stable")
    ks = key[eorder]
    kb = ks // CPC
    starts = np.searchsorted(kb, np.arange(NCORE * BANKS), side="left")
    r = np.arange(len(ks)) - starts[kb]
    s_e = r // P
    p_e = r % P
    b_e = kb % BANKS
    ke = kb // BANKS

    j_loc = j_e[eorder]
    bb_flat = np.concatenate(bbase)
    w0_flat = np.concatenate(w0)
    wn_flat = np.concatenate(wn)
    sg_e = iwoff[b_e] + s_e
    woff = j_loc - w0_flat[sg_e]
    assert (woff >= 0).all() and (woff < wn_flat[sg_e]).all()
    bld_e = bb_flat[sg_e] + woff // 2
    half_e = woff % 2

    idxs = np.zeros((NCORE, nsub_tot, P), np.int16)
    rlom = np.full((NCORE, P, n_builds), PAD_RLO, np.uint16)
    idxs[ke, sg_e, p_e] = (col[eorder] & (BKSZ - 1)).astype(np.int16)
    rlom[ke, p_e, bld_e] = ((row[eorder] & (C - 1))
                            + half_e * C).astype(np.uint16)

    # gather instructions, issued a few chunks ahead of first consumption
    instrs = []
    for b in range(BANKS):
        for (o, n) in _chop_stream(int(nsub[b])):
            issue_at = max(0, int(w0[b][o]) - LOOKAHEAD)
            instrs.append((issue_at, b, o, n))
    instrs.sort()
    schedule = []
    gi = 0
    for j in range(CPC):
        while gi < len(instrs) and instrs[gi][0] <= j:
            _, b, o, n = instrs[gi]
            schedule.append(("g", b, o, n))
            gi += 1
        schedule.append(("c", j))
    assert gi == len(instrs)

    # int16 wrap per gather instruction: idx j at [j % 16, j // 16]
    idxw16 = np.empty((NCORE, 16, nsub_tot * P // 16), np.int16)
    for ev in schedule:
        if ev[0] != "g":
            continue
        _, b, o, n = ev
        og = int(iwoff[b]) + o
        blk = idxs[:, og:og + n, :].reshape(NCORE, n * P // 16, 16)
        idxw16[:, :, og * 8:(og + n) * 8] = np.swapaxes(blk, 1, 2)
    idxw = np.broadcast_to(idxw16[:, None, :, :],
                           (NCORE, 8, 16, nsub_tot * P // 16))
    idxw = np.ascontiguousarray(idxw).reshape(NCORE, 128, nsub_tot * P // 16)

    plan = _Plan(nsub, iwoff, n_builds, bwidth, entries, schedule)
    return plan, idxw, rlom, core_of, loc_of


def _dma_gather_raw(gp, mybir, out_ap, in_ap, idxs_ap, num_idxs, num_reg,
                    elem_size, elem_step):
    """dma_gather with elem_size below the 256B transpose-mode minimum and
    single_packet=False (descriptors stream through the SWDGE ring, so
    num_idxs may exceed the 1024-desc ring; <=12288 verified on hardware).
    The non-transpose ucode accepts any elem_size as long as the row stride
    (elem_step) is a multiple of 256B; verified on hardware."""
    stride_bytes = elem_step * mybir.dt.size(in_ap.dtype)
    assert stride_bytes % 256 == 0
    _in_ap = gp.lower_ap_dma(in_ap, for_custom_bir_dma=True)
    _idxs_ap = gp.lower_ap(idxs_ap)
    _out_ap = gp.lower_ap(out_ap)
    return gp.add_instruction(
        mybir.InstDMAGatherAnt(
            name=gp.bass.get_next_instruction_name(),
            ins=[*_in_ap, _idxs_ap, gp.lower_val_access(num_reg)],
            outs=[_out_ap],
            transpose=False,
            num_idxs=num_idxs,
            elem_size=elem_size,
            stride_bytes_256=stride_bytes // 256,
            gen_mode=0,
            single_packet=False,
            queue_num=0,
            sbuf_tokens_per_rank=0,
            sbuf_free_dim_per_rank=0,
            sbuf_free_dim_pad_per_rank=0,
            sbuf_byte_offset=0,
        ))


def _build_program(plan, sim=False):
    import concourse.bacc as bacc
    import concourse.mybir as mybir
    import concourse.tile as tile
    from concourse.library_config import mlp

    f32, bf16 = mybir.dt.float32, mybir.dt.bfloat16
    i16, u16 = mybir.dt.int16, mybir.dt.uint16
    Alu = mybir.AluOpType

    nsub, iwoff, n_builds = plan.nsub, plan.iwoff, plan.n_builds
    nsub_tot = int(nsub.sum())
    nc = bacc.Bacc("TRN2", target_bir_lowering=False, debug=False,
                   enable_asserts=False, num_devices=1 if sim else NCORE)

    t_x = nc.dram_tensor("xpad", [N_NODES, XW], bf16, kind="ExternalInput")
    t_idxw = nc.dram_tensor("idxw", [128, nsub_tot * P // 16], i16,
                            kind="ExternalInput")
    t_rlom = nc.dram_tensor("rlom", [P, n_builds], u16, kind="ExternalInput")
    t_iota = nc.dram_tensor("iota", [P, 2 * C], bf16, kind="ExternalInput")
    t_disl = nc.dram_tensor("disl", [C, CPC], f32, kind="ExternalInput")
    t_out = nc.dram_tensor("out", [C, CPC, D], bf16, kind="ExternalOutput")

    bank_hi = [min((b + 1) * BKSZ, N_NODES) for b in range(BANKS)]
    OB_FLUSH = 7                  # locs per output store
    n_instr_b = [len(_chop_stream(int(nsub[b]))) for b in range(BANKS)]

    with tile.TileContext(nc) as tc:
        with (
            tc.tile_pool(name="const", bufs=1) as cpool,
            tc.tile_pool(name="gt0", bufs=min(3, n_instr_b[0])) as gp0,
            tc.tile_pool(name="gt1", bufs=min(3, n_instr_b[1])) as gp1,
            tc.tile_pool(name="gt2", bufs=min(3, n_instr_b[2])) as gp2,
            tc.tile_pool(name="gt3", bufs=min(3, n_instr_b[3])) as gp3,
            tc.tile_pool(name="gtt", bufs=8) as gtt,
            tc.tile_pool(name="ohs", bufs=10) as ohspool,
            tc.tile_pool(name="ohp", bufs=7) as ohppool,
            tc.tile_pool(name="po", bufs=6) as popool,
            tc.tile_pool(name="oha", bufs=5) as ohapool,
            tc.tile_pool(name="psum", bufs=8, space="PSUM") as ppool,
        ):
            gpools = [gp0, gp1, gp2, gp3]
            nc.gpsimd.load_library(mlp)
            iota = cpool.tile([P, 2 * C], bf16, tag="iota")
            disl = cpool.tile([C, CPC], f32, tag="disl")
            rlomu = cpool.tile([P, n_builds], u16, tag="rlomu")
            rlom = cpool.tile([P, n_builds], f32, tag="rlom")
            iw = cpool.tile([128, nsub_tot * 8], i16, tag="iw")
            ob = cpool.tile([C, CPC, D], bf16, tag="ob")

            # idx head loads first so gathers start immediately; constants
            # next so DVE/Act can start; idx bulk last
            head = [min(FRONT_SUBS + NI_BIG, int(nsub[b]))
                    for b in range(BANKS)]
            # rlom head + iota first so one-hot builds start within ~2us
            rh = min(256, n_builds)
            nc.scalar.dma_start(out=iota[:], in_=t_iota.ap())
            nc.sync.dma_start(out=rlomu[:, 0:rh], in_=t_rlom.ap()[:, 0:rh])
            nc.vector.tensor_copy(out=rlom[:, 0:rh], in_=rlomu[:, 0:rh])
            for b in range(BANKS):
                o0 = int(iwoff[b])
                nc.sync.dma_start(out=iw[:, o0 * 8:(o0 + head[b]) * 8],
                                  in_=t_idxw.ap()[:, o0 * 8:(o0 + head[b]) * 8])
            nc.scalar.dma_start(out=disl[:], in_=t_disl.ap())
            if n_builds > rh:
                nc.scalar.dma_start(out=rlomu[:, rh:], in_=t_rlom.ap()[:, rh:])
                nc.vector.tensor_copy(out=rlom[:, rh:], in_=rlomu[:, rh:])
            for b in range(BANKS):
                o0 = int(iwoff[b]) + head[b]
                o1 = int(iwoff[b]) + int(nsub[b])
                if o1 > o0:
                    nc.scalar.dma_start(out=iw[:, o0 * 8:o1 * 8],
                                        in_=t_idxw.ap()[:, o0 * 8:o1 * 8])

            num_regs = {}
            slot_map = {}
            built = {}            # build id -> (tile, slot)
            cursors = {"ohs": [None, OHB], "ohp": [None, OHB],
                       "po": [None, OHB], "oha": [None, OHB]}
            pools = {"ohs": ohspool, "ohp": ohppool, "po": popool,
                     "oha": ohapool}
            done = {"d": 0, "p": 0, "a": 0}

            def slot_of(ckey, wid):
                cur = cursors[ckey]
                if cur[1] >= OHB:
                    cur[0] = pools[ckey].tile([P, OHB, wid], bf16,
                                              tag=ckey, name=ckey)
                    cur[1] = 0
                cur[1] += 1
                return cur[0], cur[1] - 1

            def ensure_built(j):
                for (b, s, bld, half) in plan.entries[j]:
                    if bld in built:
                        continue
                    if plan.bwidth[bld] == 2:
                        tile_, slot = slot_of("ohp", 2 * C)
                        nc.vector.tensor_scalar(tile_[:, slot, :], iota[:],
                                                rlom[:, bld:bld + 1], None,
                                                Alu.is_equal)
                        built[bld] = (tile_, slot)
                        continue
                    tot = sum(done.values()) + 1
                    e = max(BUILD_W, key=lambda k: BUILD_W[k] * tot - done[k])
                    done[e] += 1
                    if e == "a":
                        # Act one-hot: Relu(1 - |iota - r|), exact on ints;
                        # Abs then in-place Relu
                        tile_, slot = slot_of("oha", C)
                        nc.scalar.activation(
                            tile_[:, slot, :], iota[:, 0:C],
                            mybir.ActivationFunctionType.Abs,
                            bias=rlom[:, bld:bld + 1], scale=-1.0)
                        nc.scalar.activation(
                            tile_[:, slot, :], tile_[:, slot, :],
                            mybir.ActivationFunctionType.Relu,
                            bias=1.0, scale=-1.0)
                    else:
                        eng = nc.gpsimd if e == "p" else nc.vector
                        tile_, slot = slot_of("po" if e == "p" else "ohs", C)
                        eng.tensor_scalar(tile_[:, slot, :], iota[:, 0:C],
                                          rlom[:, bld:bld + 1], None,
                                          Alu.is_equal)
                    built[bld] = (tile_, slot)

            ACT_LAG = 2           # chunks the pm->ob copy trails, so copies
            pending = []          # never head-of-line-block Act's builds
            j_flushed = 0

            def emit_copy(jc, pmc):
                nc.scalar.activation(ob[:, jc, :], pmc[:],
                                     mybir.ActivationFunctionType.Copy,
                                     scale=disl[:, jc:jc + 1])

            def flush(jc):
                nonlocal j_flushed
                if jc - j_flushed + 1 >= OB_FLUSH or jc == CPC - 1:
                    nc.sync.dma_start(
                        out=t_out.ap()[:, j_flushed:jc + 1, :],
                        in_=ob[:, j_flushed:jc + 1, :])
                    j_flushed = jc + 1

            for ev in plan.schedule:
                if ev[0] == "g":
                    _, b, o, n = ev
                    og = int(iwoff[b]) + o
                    if n <= NI_TAIL:
                        gt = gtt.tile([P, NI_TAIL, D], bf16, tag="gtt",
                                      name="gtt")
                    else:
                        gt = gpools[b].tile([P, NI_BIG, D], bf16, tag="gt",
                                            name="gt")
                    ni = n * P
                    if ni not in num_regs:
                        num_regs[ni] = nc.gpsimd.to_reg(ni)
                    _dma_gather_raw(
                        nc.gpsimd, mybir, gt[:, 0:n, :],
                        t_x.ap()[b * BKSZ:bank_hi[b]],
                        iw[:, og * 8:(og + n) * 8],
                        num_idxs=ni, num_reg=num_regs[ni],
                        elem_size=D, elem_step=XW)
                    for t in range(n):
                        slot_map[(b, o + t)] = (gt, t)
                    continue
                j = ev[1]
                for jj in range(j, min(j + 1 + BUILD_AHEAD, CPC)):
                    ensure_built(jj)
                pm = ppool.tile([C, D], f32, tag="pm", space="PSUM")
                tl = plan.entries[j]
                for ti, (b, s, bld, half) in enumerate(tl):
                    gt, sl = slot_map[(b, s)]
                    tile_, slot = built[bld]
                    nc.tensor.matmul(
                        pm[:],
                        lhsT=tile_[:, slot, half * C:(half + 1) * C],
                        rhs=gt[:, sl, :],
                        start=(ti == 0),
                        stop=(ti == len(tl) - 1))
                pending.append((j, pm))
                while pending and pending[0][0] <= j - ACT_LAG:
                    jc, pmc = pending.pop(0)
                    emit_copy(jc, pmc)
                    flush(jc)
            for jc, pmc in pending:
                emit_copy(jc, pmc)
                flush(jc)

    nc.compile()
    return nc


def _prepare(row, col):
    plan, idxw, rlom, core_of, loc_of = _pack_edges(row, col)
    deg = np.bincount(row, minlength=N_NODES).astype(np.float32)
    dis = np.where(deg > 0, 1.0 / np.sqrt(np.maximum(deg, 1.0)), 0.0)
    return {"build_args": (plan,), "plan": plan, "idxw": idxw, "rlom": rlom,
            "core_of": core_of, "loc_of": loc_of,
            "dis": dis.astype(np.float32)}


def _in_maps(x, prep):
    dis = prep["dis"]
    xpad = np.zeros((N_NODES, XW), ml_dtypes.bfloat16)
    xpad[:, :D] = (x * dis[:, None]).astype(ml_dtypes.bfloat16)
    iota = np.tile(np.arange(2 * C, dtype=ml_dtypes.bfloat16), (P, 1))
    dis_pad = np.zeros(NPAD, np.float32)
    dis_pad[:N_NODES] = dis
    gmap = np.zeros((NCORE, CPC), np.int64)
    gmap[prep["core_of"], prep["loc_of"]] = np.arange(NCHUNK)
    disl = dis_pad.reshape(NCHUNK, C)[gmap]                  # [NCORE, CPC, C]
    disl = np.ascontiguousarray(np.transpose(disl, (0, 2, 1)))
    return [
        {"xpad": xpad, "idxw": prep["idxw"][k],
         "rlom": prep["rlom"][k],
         "iota": iota, "disl": disl[k]}
        for k in range(NCORE)
    ]


def kernel(x, edge_index):
    from concourse import bass_utils

    x = np.asarray(x, dtype=np.float32)
    ei = np.asarray(edge_index)
    row = ei[0].astype(np.int64)
    col = ei[1].astype(np.int64)

    prep = _prepare(row, col)
    nc = _build_program(*prep["build_args"])
    in_maps = _in_maps(x, prep)
    res = bass_utils.run_bass_kernel_spmd(nc, in_maps, core_ids=list(range(NCORE)))

    gmap = np.zeros((NCORE, CPC), np.int64)
    gmap[prep["core_of"], prep["loc_of"]] = np.arange(NCHUNK)
    out_pad = np.empty((NCHUNK, C, D), np.float32)
    for k in range(NCORE):
        out_pad[gmap[k]] = np.transpose(
            res.results[k]["out"].astype(np.float32), (1, 0, 2))
    return out_pad.reshape(NPAD, D)[:N_NODES].astype(np.float32)
